# revision 30
# baseline (speedup 1.0000x reference)
"""Trainium2 Bass kernel for nn_ComplexQuantumLayer (10-qubit circuit, batch 2048).

Math: the circuit after the RX AngleEmbedding is a fixed unitary U (depends only
on `weights`), and the embedded state is a Kronecker product
  psi0[b] = (-i)^popcount(j) * m[b, j],   m[b] = kron_q [cos(x_bq/2), sin(x_bq/2)].
Folding the phase into W = diag(phase) @ U^T gives  psi = m @ W  with m REAL.
Per sample the device does two real (1024,1024) matvecs, |psi|^2, and the ten
PauliZ sums as one more matmul against a +/-1 mask matrix.

Device pipeline (per core, 256 samples = 2 partition tiles, fp16 operands):
  1. 2-level Kronecker combine from host quad-products -> m (batch, amp)
     fp16, via broadcast-AP tensor_tensor ops (lvl1 DVE, lvl2 GpSimd in
     2-chunk granules)
  2. 16 PE transposes -> mt (amp, batch) fp16 (PSUM->SBUF copies on ACT/DVE)
  3. psiT[jchunk] = Wchunk.T @ mt (8 fp16 matmuls, fp32 PSUM) x2 (re/im)
  4. ACT squares re/im into persistent fp16 planes
  5. two end-phases of back-to-back Z-mask matmuls (lhsT zero-padded to 32
     cols, which runs at full rate) accumulate partial Z sums zpA (chunks
     0-3, flushed mid-stream) and zpB (chunks 4-7); the host adds the two
     (NQ, 256) partials and transposes.
The host sends, per sample, 36 quad products of cos/sin (16 for qubits 0-3,
16 for qubits 4-7, 4 for qubits 8-9) - input-sized preprocessing only.

Schedule notes: the pp DMA is issued first so the tree starts immediately;
12 PE warm-up matmuls + chunk j=0's re-part accumulation interleaved into
the transpose phase keep the tensor engine continuously busy (its clock
ramps with busy time), making the 128-matmul main stream run back-to-back
at ~109ns each; squares ride ACT only.

Sharding: pure data parallel - batch 2048 split as 256 rows per each of the
8 cores; W (fp16, ~4.3MB) replicated per core.
"""

import numpy as np

import concourse.bass as bass
import concourse.bacc as bacc
import concourse.mybir as mybir
from concourse.bass_utils import run_bass_kernel_spmd
from concourse.masks import make_identity
from concourse.tile import TileContext

NQ = 10
DIM = 1 << NQ          # 1024
BATCH = 2048
NCORES = 8
BPC = BATCH // NCORES  # 256 rows per core
P = 128                # partitions
BT = BPC // P          # batch tiles per core = 2
KC = DIM // P          # in-amp chunks = 8
JC = DIM // P          # out-amp chunks = 8

F32 = mybir.dt.float32
F16 = mybir.dt.float16
MUL = mybir.AluOpType.mult
ADD = mybir.AluOpType.add

LAST_RESULT = None  # BassKernelResults of the most recent run (for test harness)


# ----------------------------------------------------------------------------
# Host-side preprocessing: circuit unitary from weights (numpy, ~2s)
# ----------------------------------------------------------------------------

def _build_circuit_matrix(weights: np.ndarray, dtype=np.complex128) -> np.ndarray:
    """M = U^T: the reference circuit (post-embedding) applied to identity rows."""
    w = weights.astype(np.float64)
    state = np.eye(DIM, dtype=dtype)

    def apply_1q(state, g, q):
        s = state.reshape(DIM, 1 << q, 2, -1)
        s0 = s[:, :, 0, :].copy()
        s1 = s[:, :, 1, :].copy()
        s[:, :, 0, :] = g[0, 0] * s0 + g[0, 1] * s1
        s[:, :, 1, :] = g[1, 0] * s0 + g[1, 1] * s1
        return state

    def apply_2q(state, g4, q1, q2):
        g = g4.reshape(2, 2, 2, 2)
        if q1 > q2:
            g = np.transpose(g, (1, 0, 3, 2))
            q1, q2 = q2, q1
        A = 1 << q1
        M = 1 << (q2 - q1 - 1)
        s = state.reshape(DIM, A, 2, M, 2, -1)
        blocks = [s[:, :, c, :, d, :].copy() for c in (0, 1) for d in (0, 1)]
        for a in (0, 1):
            for b in (0, 1):
                acc = None
                for c in (0, 1):
                    for d in (0, 1):
                        coef = g[a, b, c, d]
                        if coef == 0:
                            continue
                        term = coef * blocks[2 * c + d]
                        acc = term if acc is None else acc + term
                s[:, :, a, :, b, :] = 0 if acc is None else acc
        return state

    def rot_matrix(phi, theta, omega):
        ct, st = np.cos(theta / 2), np.sin(theta / 2)
        return np.array(
            [[np.exp(-0.5j * (phi + omega)) * ct, -np.exp(0.5j * (phi - omega)) * st],
             [np.exp(-0.5j * (phi - omega)) * st, np.exp(0.5j * (phi + omega)) * ct]]
        )

    CNOT = np.array([[1, 0, 0, 0], [0, 1, 0, 0], [0, 0, 0, 1], [0, 0, 1, 0]], dtype)
    I4 = np.eye(4, dtype=dtype)
    XX = np.array([[0, 0, 0, 1], [0, 0, 1, 0], [0, 1, 0, 0], [1, 0, 0, 0]], dtype)
    YY = np.array([[0, 0, 0, -1], [0, 0, 1, 0], [0, 1, 0, 0], [-1, 0, 0, 0]], dtype)

    n_layers = w.shape[0]
    for l in range(n_layers):
        wl = w[l]
        for q in range(NQ):
            state = apply_1q(state, rot_matrix(*wl[q]), q)
        for q in range(NQ):
            state = apply_2q(state, CNOT, q, (q + 1) % NQ)
        c, s_ = np.cos(wl[0, 0] / 2), np.sin(wl[0, 0] / 2)
        state = apply_2q(state, c * I4 + (-1j * s_) * XX, 0, 1)
        c, s_ = np.cos(wl[0, 1] / 2), np.sin(wl[0, 1] / 2)
        state = apply_2q(state, c * I4 + (-1j * s_) * YY, 1, 2)
        e, ec = np.exp(-0.5j * wl[0, 2]), np.exp(0.5j * wl[0, 2])
        state = apply_2q(state, np.diag(np.array([e, ec, ec, e])), 2, 3)
    return state


def _host_prepare(x: np.ndarray, weights: np.ndarray):
    M = _build_circuit_matrix(weights)
    pc = np.array([bin(k).count("1") for k in range(DIM)])
    W = ((-1j) ** pc)[:, None] * M
    wr = W.real.astype(np.float16)   # (1024, 1024) [k, n]
    wi = W.imag.astype(np.float16)

    # wt[j, p, s, c]: j = out-amp chunk, p = in-amp within chunk,
    # s in 0..7 -> (in-chunk ko=s, real), 8..15 -> (ko=s-8, imag),
    # s = 16 -> Z-mask rows: wt[j, p, 16, q] = 1 - 2*bit_q(j*128 + p)
    wr4 = wr.reshape(KC, P, JC, P).transpose(2, 1, 0, 3)  # [j, p, ko, c]
    wi4 = wi.reshape(KC, P, JC, P).transpose(2, 1, 0, 3)
    wt = np.zeros((JC, P, 17, P), dtype=np.float16)
    wt[:, :, 0:8, :] = wr4
    wt[:, :, 8:16, :] = wi4
    n = np.arange(DIM)
    zm = (1 - 2 * ((n[:, None] >> (NQ - 1 - np.arange(NQ))[None, :]) & 1)).astype(
        np.float16
    )  # (1024, 10)
    wt[:, :, 16, :NQ] = zm.reshape(JC, P, NQ)
    wt = np.ascontiguousarray(wt)

    # quad products: cols 0:16 = qubits 0-3 (digit = 8b0+4b1+2b2+b3),
    # 16:32 = qubits 4-7, 32:36 = qubits 8-9 (digit = 2b8+b9)
    xd = x.astype(np.float64)
    c = np.cos(xd / 2)
    s = np.sin(xd / 2)
    B = x.shape[0]
    pp = np.empty((B, 36), dtype=np.float32)

    def quad(qs):
        out = np.ones((B, 1))
        for q in qs:
            f = np.stack([c[:, q], s[:, q]], axis=1)
            out = (out[:, :, None] * f[:, None, :]).reshape(B, -1)
        return out

    pp[:, 0:16] = quad((0, 1, 2, 3))
    pp[:, 16:32] = quad((4, 5, 6, 7))
    pp[:, 32:36] = quad((8, 9))
    return pp, wt


# ----------------------------------------------------------------------------
# Bass kernel (per-core program; SPMD across 8 cores)
# ----------------------------------------------------------------------------

def _build_bass() -> bass.Bass:
    nc = bacc.Bacc(trn_type="TRN2")

    pp_d = nc.dram_tensor("pp", (BPC, 36), F32, kind="ExternalInput")
    wt_d = nc.dram_tensor("wt", (JC, P, 17, P), F16, kind="ExternalInput")
    out_d = nc.dram_tensor("out", (2, NQ, BPC), F32, kind="ExternalOutput")

    with TileContext(nc) as tc:
        with (
            tc.tile_pool(name="wpool", bufs=1) as w_pool,
            tc.tile_pool(name="work", bufs=1) as work_pool,
            tc.tile_pool(name="scr", bufs=2) as scr_pool,
            tc.tile_pool(name="tpsum", bufs=3, space="PSUM") as tpsum,
            tc.tile_pool(name="mpsum", bufs=2, space="PSUM") as mpsum,
            tc.tile_pool(name="zpsum", bufs=1, space="PSUM") as zpsum,
        ):
            # ---- pp DMA first (sync) so the tree can start immediately;
            # W chunk DMAs spread across sync/scalar issue queues.
            pp_sb = work_pool.tile([P, BT, 36], F32, name="pp")
            nc.sync.dma_start(pp_sb[:], pp_d.rearrange("(a p) c -> p a c", a=BT))

            w_sb = []
            for j in range(JC):
                t = w_pool.tile([P, 17, P], F16, name=f"w_{j}")
                w_sb.append(t)
            for j in range(JC):
                nc.sync.dma_start(w_sb[j][:], wt_d[j])

            identity = work_pool.tile([P, P], F16, name="identity")
            make_identity(nc, identity)

            # ---- PE warm-up: dummy matmuls start the tensor engine's
            # clock ramp while the pp DMA and tree run elsewhere.
            warm_ps = zpsum.tile([P, P], F32, name="warm")
            for _ in range(12):
                nc.tensor.matmul(warm_ps, lhsT=identity, rhs=identity,
                                 start=True, stop=True)

            # ---- 2-level Kronecker combine from quad products -> mb (fp16)
            # l1[p, dB*4 + dC] = ppB[dB] * ppC[dC]          (64 cols, fp32)
            # mb[p, dA*64 + t] = ppA[dA] * l1[t]            (1024 cols, fp16)
            l1_sb = []
            mb_sb = []
            for bt in range(BT):
                l1 = work_pool.tile([P, 64], F32, name=f"l1_{bt}")
                mb = work_pool.tile([P, DIM], F16, name=f"mb_{bt}")
                l1_sb.append(l1)
                mb_sb.append(mb)
            for bt in range(BT):
                pp = pp_sb[:, bt, :]
                dv = l1_sb[bt][:, :].rearrange("p (a b) -> p a b", a=16)
                s0 = pp[:, 32:36].unsqueeze(1).to_broadcast((P, 16, 4))
                s1 = pp[:, 16:32].unsqueeze(2).to_broadcast((P, 16, 4))
                nc.vector.tensor_tensor(dv, s1, s0, MUL)

            # lvl2 runs on GpSimd in 2-chunk granules (256 cols/op so the
            # per-op overhead amortizes); the PE transposes chunks as they
            # land (copies: ACT for tile 0, DVE for tile 1) with chunk j=0's
            # re-part accumulation riding along, keeping the whole phase
            # PE-bound.
            mt = work_pool.tile([P, KC, BPC], F16, name="mt")
            ps_r0 = mpsum.tile([P, BPC], F32, name="psr", tag="mmps")
            for g in range(4):
                for bt in range(BT):
                    dv = mb_sb[bt][:, g * 256:(g + 1) * 256].rearrange(
                        "p (a b) -> p a b", a=4)
                    s0 = l1_sb[bt][:, :].unsqueeze(1).to_broadcast((P, 4, 64))
                    s1 = pp_sb[:, bt, 4 * g:4 * g + 4].unsqueeze(2).to_broadcast(
                        (P, 4, 64))
                    nc.gpsimd.tensor_tensor(dv, s1, s0, MUL)
                for k in (2 * g, 2 * g + 1):
                    for bt in range(BT):
                        tp = tpsum.tile([P, P], F16, name="tp", tag="tp")
                        nc.tensor.transpose(
                            tp, mb_sb[bt][:, k * P:(k + 1) * P], identity)
                        if bt == 0:
                            nc.scalar.copy(mt[:, k, 0:P], tp)
                        else:
                            nc.vector.tensor_copy(mt[:, k, P:BPC], tp)
                    nc.tensor.matmul(
                        ps_r0, lhsT=w_sb[0][:, k, :], rhs=mt[:, k, :],
                        start=(k == 0), stop=(k == KC - 1),
                    )

            # ---- per out-chunk: matmuls + |psi|^2 into persistent fp16
            # planes. Z-mask matmuls accumulate into two PSUM tiles: zpA
            # (chunks 0-3) is flushed to DRAM mid-stream, zpB (chunks 4-7)
            # at the end; the host adds the two partial Z sums.
            p_sb = work_pool.tile([P, JC, 2, BPC], F16, name="p_sb")
            zpA = zpsum.tile([32, BPC], F32, name="zpA")
            zpB = zpsum.tile([32, BPC], F32, name="zpB")
            zoutA = work_pool.tile([NQ, BPC], F32, name="zoutA")
            zoutB = work_pool.tile([NQ, BPC], F32, name="zoutB")

            def z_phase(jlo, jhi, zp):
                for j in range(jlo, jhi):
                    for c in range(2):
                        nc.tensor.matmul(
                            zp, lhsT=w_sb[j][:, 16, 0:32],
                            rhs=p_sb[:, j, c, :],
                            start=(j == jlo and c == 0),
                            stop=(j == jhi - 1 and c == 1),
                            skip_group_check=True,
                        )

            for j in range(JC):
                if j == 0:
                    ps_r = ps_r0
                else:
                    ps_r = mpsum.tile([P, BPC], F32, name="psr", tag="mmps")
                    for k in range(KC):
                        nc.tensor.matmul(
                            ps_r, lhsT=w_sb[j][:, k, :], rhs=mt[:, k, :],
                            start=(k == 0), stop=(k == KC - 1),
                        )
                if j == 5:
                    z_phase(0, 4, zpA)
                    nc.vector.tensor_copy(zoutA[:], zpA[0:NQ, :])
                    nc.sync.dma_start(out_d[0], zoutA[:])
                ps_i = mpsum.tile([P, BPC], F32, name="psi", tag="mmps")
                for k in range(KC):
                    nc.tensor.matmul(
                        ps_i, lhsT=w_sb[j][:, 8 + k, :], rhs=mt[:, k, :],
                        start=(k == 0), stop=(k == KC - 1),
                    )
                nc.scalar.square(p_sb[:, j, 0, :], ps_r)
                nc.scalar.square(p_sb[:, j, 1, :], ps_i)

            z_phase(4, JC, zpB)
            nc.vector.tensor_copy(zoutB[:], zpB[0:NQ, :])
            nc.sync.dma_start(out_d[1], zoutB[:])

    nc.finalize()
    return nc


# ----------------------------------------------------------------------------
# Entry point
# ----------------------------------------------------------------------------

def kernel(x: np.ndarray, weights: np.ndarray, _trace: bool = False) -> np.ndarray:
    global LAST_RESULT
    x = np.asarray(x, dtype=np.float32)
    weights = np.asarray(weights, dtype=np.float32)

    pp, wt = _host_prepare(x, weights)

    nc = _build_bass()
    in_maps = [
        {"pp": np.ascontiguousarray(pp[i * BPC:(i + 1) * BPC]), "wt": wt}
        for i in range(NCORES)
    ]
    res = run_bass_kernel_spmd(
        nc, in_maps, core_ids=list(range(NCORES)), trace=_trace
    )
    LAST_RESULT = res
    out = np.concatenate(
        [(np.asarray(r["out"][0]) + np.asarray(r["out"][1])).T
         for r in res.results], axis=0)
    return np.ascontiguousarray(out).astype(np.float32)
